# revision 20
# baseline (speedup 1.0000x reference)
"""Associative-embedding (AE) loss kernel for Trainium2, 8 NeuronCores.

Strategy (data-parallel over batch, per the sharding hint):
  - B=8 images, 8 cores -> one image per core.
  - Per core: gather the 30x17 tag values tags[k, idx[m,k]] with indirect
    (gather) DMAs -- only 510 x 4B of the 17MB tag map is touched. The HW
    DGE consumes ONE index per output partition row, so the k<16 indices
    are loaded straight from DRAM into a [120, 4] layout (partition
    p = 4m + k//4, column c = k & 3) and gathered with 4 calls of [120, 1];
    the k=16 column is one [30, 1] call. Row offsets k*HW are added
    in-layout via k = (4p + c) & 15.
  - All per-person reductions run directly on the [120, 4] gather layout;
    group-of-4-partition sums go through one tiny PE matmul with a
    0/1 selection matrix, so the gathered data is never re-laid-out.
  - push pair sum = pv^T E pv with two PE matmuls (E symmetric), which
    also yields pull_sum = pv^T pull_pp and n = pv^T pv for free.
  - Each core writes its per-image partial; the batch sum (the unshard of a
    data-parallel loss) happens on the host over the 8 pairs.
    (An ncfw AllReduce of the two scalars was measured at 70-150us on this
    stack -- 5x the rest of the kernel -- so it is deliberately avoided.)
"""

import numpy as np

import concourse.bass as bass
import concourse.bacc as bacc
import concourse.tile as tile
from concourse import mybir
from concourse.bass_utils import run_bass_kernel_spmd

B, K, HW, M = 8, 17, 262144, 30
NCORES = 8
MP = 32  # person dim padded (transpose block / matmul contraction)
KL = 16  # k<16 columns gathered via the [120, 4] layout
GP, GC = 120, 4

F32 = mybir.dt.float32
I32 = mybir.dt.int32
AX = mybir.AxisListType
OP = mybir.AluOpType
ACT = mybir.ActivationFunctionType


def build_nc(finalize=True):
    nc = bacc.Bacc(None, num_devices=NCORES)
    tags = nc.declare_dram_parameter("tags", [K, HW], F32, isOutput=False)
    kp = nc.declare_dram_parameter("kp", [M, K, 2], I32, isOutput=False)
    out = nc.declare_dram_parameter("out", [1, 2], F32, isOutput=True)

    with tile.TileContext(nc) as tc:
        with (
            tc.tile_pool(name="sb", bufs=1) as sb,
            tc.tile_pool(name="ps", bufs=1, space="PSUM") as ps,
        ):
            # warm the ACT table early so Square/Exp don't pay the 1.3us
            # table load on the critical path
            warm = sb.tile([1, 1], F32)
            nc.vector.memset(warm[:], 0.0)
            nc.scalar.activation(out=warm[:], in_=warm[:], func=ACT.Square)

            # ---------------- index loads + gather ----------------
            icols = sb.tile([GP, GC], I32)
            nc.sync.dma_start(out=icols[:, :], in_=kp[:, 0:KL, 0])
            kp16 = sb.tile([M, 2], I32)
            nc.sync.dma_start(out=kp16[:, :], in_=kp[:, KL, :])
            viscols = sb.tile([GP, GC], I32)
            nc.scalar.dma_start(out=viscols[:, :], in_=kp[:, 0:KL, 1])

            # k*HW in-layout: k = (4p + c) & 15
            kofs = sb.tile([GP, GC], I32)
            nc.gpsimd.iota(kofs[:], pattern=[[1, GC]], base=0, channel_multiplier=GC)
            nc.vector.tensor_scalar(
                out=kofs[:], in0=kofs[:], scalar1=KL - 1, scalar2=None, op0=OP.bitwise_and
            )
            nc.vector.tensor_scalar(
                out=kofs[:], in0=kofs[:], scalar1=HW, scalar2=None, op0=OP.mult
            )
            nc.vector.tensor_tensor(out=icols[:], in0=icols[:], in1=kofs[:], op=OP.add)
            flat16 = sb.tile([M, 1], I32)
            nc.vector.tensor_scalar(
                out=flat16[:], in0=kp16[:, 0:1], scalar1=KL * HW, scalar2=None, op0=OP.add
            )

            g16 = sb.tile([M, 1], F32)
            nc.gpsimd.indirect_dma_start(
                out=g16[:],
                out_offset=None,
                in_=tags[:, :],
                in_offset=bass.IndirectOffsetOnAxis(ap=flat16[:], axis=1),
            )
            gcols = sb.tile([GP, GC], F32)
            for c in range(GC):
                nc.gpsimd.indirect_dma_start(
                    out=gcols[:, c : c + 1],
                    out_offset=None,
                    in_=tags[:, :],
                    in_offset=bass.IndirectOffsetOnAxis(ap=icols[:, c : c + 1], axis=1),
                )

            # ------- constants (built on GpSimd while gathers drain) -------
            # sel[p, m] = 1 iff p//4 == m   (group-of-4 partition sums)
            sel = sb.tile([GP, MP], F32)
            nc.vector.memset(sel[:], 1.0)
            nc.gpsimd.affine_select(
                out=sel[:], in_=sel[:], pattern=[[-4, MP]],
                compare_op=OP.is_ge, fill=0.0, base=0, channel_multiplier=1,
            )
            nc.gpsimd.affine_select(
                out=sel[:], in_=sel[:], pattern=[[4, MP]],
                compare_op=OP.is_ge, fill=0.0, base=3, channel_multiplier=-1,
            )
            # selT[m, q] = sel[q, m]   (spread person values back to groups)
            selT = sb.tile([MP, GP], F32)
            nc.vector.memset(selT[:], 1.0)
            nc.gpsimd.affine_select(
                out=selT[:], in_=selT[:], pattern=[[1, GP]],
                compare_op=OP.is_ge, fill=0.0, base=0, channel_multiplier=-4,
            )
            nc.gpsimd.affine_select(
                out=selT[:], in_=selT[:], pattern=[[-1, GP]],
                compare_op=OP.is_ge, fill=0.0, base=3, channel_multiplier=4,
            )
            # identity (PE transpose + injecting the k=16 [30,1] columns)
            ident = sb.tile([MP, MP], F32)
            nc.vector.memset(ident[:], 0.0)
            nc.gpsimd.affine_select(
                out=ident[:], in_=ident[:], pattern=[[-1, MP]],
                compare_op=OP.not_equal, fill=1.0, base=0, channel_multiplier=1,
            )
            id30 = ident[0:M, :]

            # ---------------- early (gather-independent) ----------------
            mask_cols = sb.tile([GP, GC], F32)
            nc.vector.tensor_scalar(
                out=mask_cols[:], in0=viscols[:], scalar1=0, scalar2=None, op0=OP.is_gt
            )
            crow = sb.tile([GP, 1], F32)
            nc.vector.tensor_reduce(out=crow[:], in_=mask_cols[:], axis=AX.X, op=OP.add)
            mask16 = sb.tile([M, 1], F32)
            nc.vector.tensor_scalar(
                out=mask16[:], in0=kp16[:, 1:2], scalar1=0, scalar2=None, op0=OP.is_gt
            )
            # cnt[m] = sum_k mask: group sums + k=16 column via PSUM accum
            cnt_ps = ps.tile([MP, 1], F32)
            nc.tensor.matmul(out=cnt_ps[:], lhsT=sel[:], rhs=crow[:], start=True, stop=False)
            nc.tensor.matmul(out=cnt_ps[:], lhsT=id30, rhs=mask16[:], start=False, stop=True)
            inv = sb.tile([MP, 1], F32)
            nc.vector.tensor_scalar(
                out=inv[:], in0=cnt_ps[:], scalar1=1.0, scalar2=None, op0=OP.max
            )
            nc.vector.reciprocal(out=inv[:], in_=inv[:])
            pvalid = sb.tile([MP, 1], F32)
            nc.vector.tensor_scalar(
                out=pvalid[:], in0=cnt_ps[:], scalar1=0.0, scalar2=None, op0=OP.is_gt
            )

            # n-dependent epilogue factors (n = pv^T pv):
            #   t[0] = 1/max(n,1), t[1] = 1/max(n^2-n,1), t[3] = (n>1)*0.5
            n_ps = ps.tile([1, 1], F32)
            nc.tensor.matmul(out=n_ps[:], lhsT=pvalid[:], rhs=pvalid[:], start=True, stop=True)
            t = sb.tile([1, 4], F32)
            nc.vector.tensor_copy(out=t[0:1, 0:1], in_=n_ps[:])
            n_ap = t[0:1, 0:1]
            nc.vector.tensor_tensor(out=t[0:1, 1:2], in0=n_ap, in1=n_ap, op=OP.mult)
            nc.vector.tensor_tensor(out=t[0:1, 1:2], in0=t[0:1, 1:2], in1=n_ap, op=OP.subtract)
            nc.vector.tensor_scalar(
                out=t[0:1, 3:4], in0=n_ap, scalar1=1.0, scalar2=0.5, op0=OP.is_gt, op1=OP.mult
            )
            nc.vector.tensor_scalar(
                out=t[0:1, 0:2], in0=t[0:1, 0:2], scalar1=1.0, scalar2=None, op0=OP.max
            )
            nc.vector.reciprocal(out=t[0:1, 0:2], in_=t[0:1, 0:2])

            # ---------------- g-dependent chain ----------------
            # mean = (group sums of g*mask) * inv
            gm = sb.tile([GP, GC], F32)
            nc.vector.tensor_tensor(out=gm[:], in0=gcols[:], in1=mask_cols[:], op=OP.mult)
            grow = sb.tile([GP, 1], F32)
            nc.vector.tensor_reduce(out=grow[:], in_=gm[:], axis=AX.X, op=OP.add)
            g16m = sb.tile([M, 1], F32)
            nc.vector.tensor_tensor(out=g16m[:], in0=g16[:], in1=mask16[:], op=OP.mult)
            sum_ps = ps.tile([MP, 1], F32)
            nc.tensor.matmul(out=sum_ps[:], lhsT=sel[:], rhs=grow[:], start=True, stop=False)
            nc.tensor.matmul(out=sum_ps[:], lhsT=id30, rhs=g16m[:], start=False, stop=True)
            mean = sb.tile([MP, 1], F32)
            nc.vector.tensor_tensor(out=mean[:], in0=sum_ps[:], in1=inv[:], op=OP.mult)

            # -- push branch: E = exp(-(mean_i - mean_j)^2), push = pv^T E pv
            mT_ps = ps.tile([MP, MP], F32)
            nc.tensor.transpose(
                out=mT_ps[:], in_=mean[:, 0:1].to_broadcast([MP, MP]), identity=ident[:]
            )
            d2 = sb.tile([MP, MP], F32)
            nc.scalar.activation(
                out=d2[:], in_=mT_ps[:], func=ACT.Square, bias=mean[:, 0:1], scale=-1.0
            )
            e = sb.tile([MP, MP], F32)
            nc.scalar.activation(out=e[:], in_=d2[:], func=ACT.Exp, bias=0.0, scale=-1.0)
            stacked = sb.tile([MP, 2], F32)
            epv_ps = ps.tile([MP, 1], F32)
            nc.tensor.matmul(out=epv_ps[:], lhsT=e[:], rhs=pvalid[:], start=True, stop=True)
            nc.scalar.copy(out=stacked[:, 1:2], in_=epv_ps[:])

            # -- pull branch: pull_pp = sum_k mask*(g-mean)^2 * inv
            mcols_ps = ps.tile([GP, 1], F32)
            nc.tensor.matmul(out=mcols_ps[:], lhsT=selT[:], rhs=mean[:], start=True, stop=True)
            mcols = sb.tile([GP, 1], F32)
            nc.scalar.copy(out=mcols[:], in_=mcols_ps[:])
            dev = sb.tile([GP, GC], F32)
            nc.vector.tensor_scalar(
                out=dev[:], in0=gcols[:], scalar1=mcols[:, 0:1], scalar2=None, op0=OP.subtract
            )
            nc.vector.tensor_tensor(out=dev[:], in0=dev[:], in1=mask_cols[:], op=OP.mult)
            dsq = sb.tile([GP, GC], F32)
            nc.vector.tensor_tensor(out=dsq[:], in0=dev[:], in1=dev[:], op=OP.mult)
            dsr = sb.tile([GP, 1], F32)
            nc.vector.tensor_reduce(out=dsr[:], in_=dsq[:], axis=AX.X, op=OP.add)
            dev16 = sb.tile([M, 1], F32)
            nc.vector.tensor_scalar(
                out=dev16[:], in0=g16[:], scalar1=mean[0:M, 0:1], scalar2=None, op0=OP.subtract
            )
            nc.vector.tensor_tensor(out=dev16[:], in0=dev16[:], in1=mask16[:], op=OP.mult)
            dsq16 = sb.tile([M, 1], F32)
            nc.vector.tensor_tensor(out=dsq16[:], in0=dev16[:], in1=dev16[:], op=OP.mult)
            # note: (dev*mask)^2 == mask*dev^2 since mask is 0/1
            pull_ps = ps.tile([MP, 1], F32)
            nc.tensor.matmul(out=pull_ps[:], lhsT=sel[:], rhs=dsr[:], start=True, stop=False)
            nc.tensor.matmul(out=pull_ps[:], lhsT=id30, rhs=dsq16[:], start=False, stop=True)
            nc.vector.tensor_tensor(
                out=stacked[:, 0:1], in0=pull_ps[:], in1=inv[:], op=OP.mult
            )

            # -- contract with pv: [pull_sum, push_sum] = pv^T [pull_pp | Epv]
            S_ps = ps.tile([1, 2], F32)
            nc.tensor.matmul(out=S_ps[:], lhsT=pvalid[:], rhs=stacked[:], start=True, stop=True)
            S = sb.tile([1, 2], F32)
            nc.scalar.copy(out=S[:], in_=S_ps[:])

            # epilogue: pull = pull_sum/max(n,1);
            #           push = (n>1) * push_sum/max(n^2-n,1) * 0.5
            res = sb.tile([1, 2], F32)
            nc.vector.tensor_tensor(
                out=res[0:1, 0:1], in0=S[0:1, 0:1], in1=t[0:1, 0:1], op=OP.mult
            )
            nc.vector.tensor_tensor(
                out=res[0:1, 1:2], in0=S[0:1, 1:2], in1=t[0:1, 1:2], op=OP.mult
            )
            nc.vector.tensor_tensor(
                out=res[0:1, 1:2], in0=res[0:1, 1:2], in1=t[0:1, 3:4], op=OP.mult
            )

            # per-core partial (pull_b, push_b) -> DRAM
            nc.sync.dma_start(out=out[:, :], in_=res[:, :])

    if finalize:
        nc.finalize()
    return nc


_NC_CACHE = None


def _get_nc():
    global _NC_CACHE
    if _NC_CACHE is None:
        _NC_CACHE = build_nc()
    return _NC_CACHE


def make_in_maps(tags, keypoint_indices):
    tags = np.ascontiguousarray(np.asarray(tags, dtype=np.float32))
    kp = np.ascontiguousarray(np.asarray(keypoint_indices, dtype=np.int32))
    assert tags.shape == (B, K, HW), tags.shape
    assert kp.shape == (B, M, K, 2), kp.shape
    return [{"tags": tags[i], "kp": kp[i]} for i in range(NCORES)]


def kernel(tags, keypoint_indices, **run_kwargs):
    nc = _get_nc()
    in_maps = make_in_maps(tags, keypoint_indices)
    r = run_bass_kernel_spmd(nc, in_maps, core_ids=list(range(NCORES)), **run_kwargs)
    parts = np.stack(
        [np.asarray(r.results[i]["out"], dtype=np.float32)[0] for i in range(NCORES)]
    )  # [8, 2] per-image (pull, push)
    pull = np.float32(parts[:, 0].sum(dtype=np.float32))
    push = np.float32(parts[:, 1].sum(dtype=np.float32))
    return (np.asarray(pull), np.asarray(push))


# revision 21
# speedup vs baseline: 1.1527x; 1.1527x over previous
"""Associative-embedding (AE) loss kernel for Trainium2, 8 NeuronCores.

Strategy (data-parallel over batch, per the sharding hint):
  - B=8 images, 8 cores -> one image per core.
  - Per core: gather the 30x17 tag values tags[k, idx[m,k]] with indirect
    (gather) DMAs -- only 510 x 4B of the 17MB tag map is touched. The HW
    DGE consumes ONE index per output partition row, so the k<16 indices
    are loaded straight from DRAM into a [120, 4] layout and gathered with
    4 calls of [120, 1]; the k=16 column is one [30, 1] call. The k*HW row
    offset is added in-layout via k = (4p + c) & 15.
  - Per-person DVE reductions run in the [30, 17] layout (one small
    SBUF->SBUF DMA restores it); the push pair sum is computed as
    pv^T E pv with two tiny PE matmuls (E = exp(-(mean_i - mean_j)^2) is
    symmetric), which also yields pull_sum = pv^T pull_pp and n = pv^T pv.
  - Each core writes its per-image partial; the batch sum (the unshard of a
    data-parallel loss) happens on the host over the 8 pairs.
    (An ncfw AllReduce of the two scalars was measured at 70-150us on this
    stack -- 5x the rest of the kernel -- so it is deliberately avoided.)
"""

import numpy as np

import concourse.bass as bass
import concourse.bacc as bacc
import concourse.tile as tile
from concourse import mybir
from concourse.bass_utils import run_bass_kernel_spmd

B, K, HW, M = 8, 17, 262144, 30
NCORES = 8
MP = 32  # person dim padded to the DVE stream-transpose block size
KL = 16  # k<16 columns gathered via the [120, 4] layout
GP, GC = 120, 4

F32 = mybir.dt.float32
I32 = mybir.dt.int32
AX = mybir.AxisListType
OP = mybir.AluOpType
ACT = mybir.ActivationFunctionType


def build_nc(finalize=True):
    nc = bacc.Bacc(None, num_devices=NCORES)
    tags = nc.declare_dram_parameter("tags", [K, HW], F32, isOutput=False)
    kp = nc.declare_dram_parameter("kp", [M, K, 2], I32, isOutput=False)
    out = nc.declare_dram_parameter("out", [1, 2], F32, isOutput=True)

    with tile.TileContext(nc) as tc:
        with (
            tc.tile_pool(name="sb", bufs=1) as sb,
            tc.tile_pool(name="ps", bufs=1, space="PSUM") as ps,
        ):
            # index loads: icols on the SP HWDGE ring, kp_t on the ACT ring
            # (both in flight together); warm the ACT table right after the
            # kp_t trigger so Square/Exp don't pay the 1.3us load later.
            icols = sb.tile([GP, GC], I32)
            nc.sync.dma_start(out=icols[:, :], in_=kp[:, 0:KL, 0])
            kp_t = sb.tile([M, K, 2], I32)
            nc.scalar.dma_start(out=kp_t[:], in_=kp[:, :, :])
            warm = sb.tile([1, 1], F32)
            nc.vector.memset(warm[:], 0.0)
            nc.scalar.activation(out=warm[:], in_=warm[:], func=ACT.Square)
            vis = kp_t[:, :, 1]

            # k*HW in-layout: k = (4p + c) & 15
            kofs = sb.tile([GP, GC], I32)
            nc.gpsimd.iota(kofs[:], pattern=[[1, GC]], base=0, channel_multiplier=GC)
            nc.vector.tensor_scalar(
                out=kofs[:], in0=kofs[:], scalar1=KL - 1, scalar2=None, op0=OP.bitwise_and
            )
            nc.vector.tensor_scalar(
                out=kofs[:], in0=kofs[:], scalar1=HW, scalar2=None, op0=OP.mult
            )
            nc.vector.tensor_tensor(out=icols[:], in0=icols[:], in1=kofs[:], op=OP.add)

            # the 4 bulk gathers go first on the Q7 queue; the k=16 gather
            # (whose index needs kp_t) goes last
            gcols = sb.tile([GP, GC], F32)
            for c in range(GC):
                nc.gpsimd.indirect_dma_start(
                    out=gcols[:, c : c + 1],
                    out_offset=None,
                    in_=tags[:, :],
                    in_offset=bass.IndirectOffsetOnAxis(ap=icols[:, c : c + 1], axis=1),
                )
            flat16 = sb.tile([M, 1], I32)
            nc.vector.tensor_scalar(
                out=flat16[:], in0=kp_t[:, KL, 0:1], scalar1=KL * HW, scalar2=None, op0=OP.add
            )
            g = sb.tile([MP, K], F32)
            nc.gpsimd.indirect_dma_start(
                out=g[:M, KL:K],
                out_offset=None,
                in_=tags[:, :],
                in_offset=bass.IndirectOffsetOnAxis(ap=flat16[:], axis=1),
            )
            # restore the [30, 17] person-major layout
            nc.sync.dma_start(out=g[:M, 0:KL], in_=gcols[:, :])

            # ---------- early compute (independent of gathered data) ----------
            mask = sb.tile([MP, K], F32)
            nc.vector.memset(mask[:], 0.0)
            nc.vector.tensor_scalar(
                out=mask[:M, :], in0=vis, scalar1=0, scalar2=None, op0=OP.is_gt
            )
            cnt = sb.tile([MP, 1], F32)
            nc.vector.tensor_reduce(out=cnt[:], in_=mask[:], axis=AX.X, op=OP.add)
            inv = sb.tile([MP, 1], F32)
            nc.vector.tensor_scalar(
                out=inv[:], in0=cnt[:], scalar1=1.0, scalar2=None, op0=OP.max
            )
            nc.vector.reciprocal(out=inv[:], in_=inv[:])
            pvalid = sb.tile([MP, 1], F32)
            nc.vector.tensor_scalar(
                out=pvalid[:], in0=cnt[:], scalar1=0.0, scalar2=None, op0=OP.is_gt
            )

            # n-dependent epilogue factors (n = pv^T pv), all hidden under
            # the gathers: t[0]=1/max(n,1), t[1]=1/max(n^2-n,1), t[3]=(n>1)*0.5
            n_ps = ps.tile([1, 1], F32)
            nc.tensor.matmul(out=n_ps[:], lhsT=pvalid[:], rhs=pvalid[:], start=True, stop=True)
            t = sb.tile([1, 4], F32)
            nc.vector.tensor_copy(out=t[0:1, 0:1], in_=n_ps[:])
            n_ap = t[0:1, 0:1]
            nc.vector.tensor_tensor(out=t[0:1, 1:2], in0=n_ap, in1=n_ap, op=OP.mult)
            nc.vector.tensor_tensor(out=t[0:1, 1:2], in0=t[0:1, 1:2], in1=n_ap, op=OP.subtract)
            nc.vector.tensor_scalar(
                out=t[0:1, 3:4], in0=n_ap, scalar1=1.0, scalar2=0.5, op0=OP.is_gt, op1=OP.mult
            )
            nc.vector.tensor_scalar(
                out=t[0:1, 0:2], in0=t[0:1, 0:2], scalar1=1.0, scalar2=None, op0=OP.max
            )
            nc.vector.reciprocal(out=t[0:1, 0:2], in_=t[0:1, 0:2])

            stacked = sb.tile([MP, 2], F32)
            nc.vector.memset(stacked[:, 0:1], 0.0)

            # ---------------- g-dependent chain ----------------
            # mean = sum(g*mask) * inv (0 on pad rows)
            gm = sb.tile([MP, K], F32)
            mean = sb.tile([MP, 1], F32)
            nc.vector.memset(mean[:], 0.0)
            nc.vector.tensor_tensor(out=gm[:M], in0=g[:M], in1=mask[:M], op=OP.mult)
            nc.vector.tensor_reduce(out=mean[:M], in_=gm[:M], axis=AX.X, op=OP.add)
            nc.vector.tensor_tensor(out=mean[:M], in0=mean[:M], in1=inv[:M], op=OP.mult)

            # pull per person: sum(mask*(g-mean)^2) * inv  (mask^2 == mask)
            dev = sb.tile([MP, K], F32)
            nc.vector.tensor_scalar(
                out=dev[:M], in0=g[:M], scalar1=mean[:M, 0:1], scalar2=None, op0=OP.subtract
            )
            nc.vector.tensor_tensor(out=dev[:M], in0=dev[:M], in1=mask[:M], op=OP.mult)
            dsq = sb.tile([MP, K], F32)
            spull = sb.tile([MP, 1], F32)
            nc.vector.tensor_tensor(out=dsq[:M], in0=dev[:M], in1=dev[:M], op=OP.mult)
            nc.vector.tensor_reduce(out=spull[:M], in_=dsq[:M], axis=AX.X, op=OP.add)
            nc.vector.tensor_tensor(
                out=stacked[:M, 0:1], in0=spull[:M], in1=inv[:M], op=OP.mult
            )

            # push: meanT via DVE 32x32 transpose of the broadcast column,
            # then E = exp(-(mean - meanT)^2) on ACT, then pv^T E pv on PE
            meanB = sb.tile([MP, MP], F32)
            nc.vector.tensor_copy(out=meanB[:], in_=mean[:, 0:1].to_broadcast([MP, MP]))
            meanT = sb.tile([MP, MP], F32)
            nc.vector.transpose(out=meanT[:], in_=meanB[:])
            d2 = sb.tile([MP, MP], F32)
            nc.scalar.activation(
                out=d2[:], in_=meanT[:], func=ACT.Square, bias=mean[:, 0:1], scale=-1.0
            )
            e = sb.tile([MP, MP], F32)
            nc.scalar.activation(out=e[:], in_=d2[:], func=ACT.Exp, bias=0.0, scale=-1.0)
            epv_ps = ps.tile([MP, 1], F32)
            nc.tensor.matmul(out=epv_ps[:], lhsT=e[:], rhs=pvalid[:], start=True, stop=True)
            nc.scalar.copy(out=stacked[:, 1:2], in_=epv_ps[:])

            # [pull_sum, push_sum] = pv^T [pull_pp | Epv]
            S_ps = ps.tile([1, 2], F32)
            nc.tensor.matmul(out=S_ps[:], lhsT=pvalid[:], rhs=stacked[:], start=True, stop=True)
            S = sb.tile([1, 2], F32)
            nc.scalar.copy(out=S[:], in_=S_ps[:])

            # epilogue: pull = pull_sum/max(n,1);
            #           push = (n>1) * push_sum/max(n^2-n,1) * 0.5
            res = sb.tile([1, 2], F32)
            nc.vector.tensor_tensor(
                out=res[0:1, 0:1], in0=S[0:1, 0:1], in1=t[0:1, 0:1], op=OP.mult
            )
            nc.vector.tensor_tensor(
                out=res[0:1, 1:2], in0=S[0:1, 1:2], in1=t[0:1, 1:2], op=OP.mult
            )
            nc.vector.tensor_tensor(
                out=res[0:1, 1:2], in0=res[0:1, 1:2], in1=t[0:1, 3:4], op=OP.mult
            )

            # per-core partial (pull_b, push_b) -> DRAM
            nc.sync.dma_start(out=out[:, :], in_=res[:, :])

    if finalize:
        nc.finalize()
    return nc


_NC_CACHE = None


def _get_nc():
    global _NC_CACHE
    if _NC_CACHE is None:
        _NC_CACHE = build_nc()
    return _NC_CACHE


def make_in_maps(tags, keypoint_indices):
    tags = np.ascontiguousarray(np.asarray(tags, dtype=np.float32))
    kp = np.ascontiguousarray(np.asarray(keypoint_indices, dtype=np.int32))
    assert tags.shape == (B, K, HW), tags.shape
    assert kp.shape == (B, M, K, 2), kp.shape
    return [{"tags": tags[i], "kp": kp[i]} for i in range(NCORES)]


def kernel(tags, keypoint_indices, **run_kwargs):
    nc = _get_nc()
    in_maps = make_in_maps(tags, keypoint_indices)
    r = run_bass_kernel_spmd(nc, in_maps, core_ids=list(range(NCORES)), **run_kwargs)
    parts = np.stack(
        [np.asarray(r.results[i]["out"], dtype=np.float32)[0] for i in range(NCORES)]
    )  # [8, 2] per-image (pull, push)
    pull = np.float32(parts[:, 0].sum(dtype=np.float32))
    push = np.float32(parts[:, 1].sum(dtype=np.float32))
    return (np.asarray(pull), np.asarray(push))
